# Initial kernel scaffold
#
"""AttentionalPropagation kernel for Trainium2 (Bass/Tile), 8-core SPMD.

x: [B=64, C=512, L=4096] f32.  Per location l: self-attention over the B axis
(q=k=v, head dim C), out = x + msg.  Sharded over L: each of 8 cores handles
L/8 = 512 locations.

Per-core dataflow (block of LB=64 locations, pairs of 2 locations packed to
fill the 128-wide PE array):
  - DMA x in "qT" layout: one [c=128, ci=4, b=64, l=LB] f32 tile (256B runs)
  - cast f32 -> bf16 with (b,l)->(l,b) permute on GPSIMD (idle engine), so
    each location-pair is a contiguous 128-column matmul operand
  - mm1: scores pair [128,128] = qT^T @ qT accumulated over 4 C-chunks (PE)
  - exp(scores/sqrt(C)) on the two diagonal 64x64 blocks into block-diag E
    (off-diagonal zeros persist in 4 rotating E tiles, zeroed once) (ACT)
  - rowsum over full E rows -> reciprocal (DVE, zeros harmless)
  - q_BC: 4 identity-matmul transposes -> [128(j,b), 512(c)] psum (PE),
    copy-cast to bf16 sbuf (DVE)
  - mm2: msg pair [128,512] = E(block-diag) @ q_BC, one K=128 N=512 mm (PE)
  - copy psum->sbuf bf16 with per-partition scale = 1/rowsum (ACT)
  - 4 identity-matmul transposes back to [c, (j,b)] psum (PE), one merged
    DVE add into the f32 x tile in place, DMA the updated tile out
"""

import numpy as np

B, C, L_FULL, N_CORES = 64, 512, 4096, 8
LS = L_FULL // N_CORES  # 512 locations per core
LB = 64                 # locations per block
N_BLK = LS // LB        # 8
N_PAIR = LB // 2        # 32 pairs per block
CCH = C // 128          # 4 c-chunks
SCALE = 1.0 / float(C) ** 0.5


def build_nc():
    from contextlib import ExitStack

    import concourse.bass as bass
    import concourse.mybir as mybir
    from concourse.masks import make_identity
    from concourse.tile import TileContext

    f32 = mybir.dt.float32
    bf16 = mybir.dt.bfloat16
    AF = mybir.ActivationFunctionType

    nc = bass.Bass()
    x = nc.dram_tensor("x", [B, C, LS], f32, kind="ExternalInput")
    y = nc.dram_tensor("y", [B, C, LS], f32, kind="ExternalOutput")

    with ExitStack() as ctx:
        tc = ctx.enter_context(TileContext(nc))
        const = ctx.enter_context(tc.tile_pool(name="const", bufs=1))
        xt_pool = ctx.enter_context(tc.tile_pool(name="xt", bufs=2))
        qt_pool = ctx.enter_context(tc.tile_pool(name="qt", bufs=2))
        sm_pool = ctx.enter_context(tc.tile_pool(name="sm", bufs=6))
        ps_s_pool = ctx.enter_context(tc.tile_pool(name="ps_s", bufs=2, space="PSUM"))
        ps_t_pool = ctx.enter_context(tc.tile_pool(name="ps_t", bufs=2, space="PSUM"))
        ps_m_pool = ctx.enter_context(tc.tile_pool(name="ps_m", bufs=2, space="PSUM"))
        ps_o_pool = ctx.enter_context(tc.tile_pool(name="ps_o", bufs=2, space="PSUM"))

        ident = const.tile([128, 128], bf16)
        make_identity(nc, ident)

        for blk in range(N_BLK):
            l0 = blk * LB
            # one big f32 tile per block: [c, ci, b, l]; doubles as out staging
            xt_t = xt_pool.tile([128, CCH, B, LB], f32, name="xt", tag="xt")
            qt = []
            for ci in range(CCH):
                nc.sync.dma_start(
                    out=xt_t[:, ci],
                    in_=x[:, ci * 128 : (ci + 1) * 128, l0 : l0 + LB].rearrange(
                        "b c l -> c b l"
                    ),
                )
                qt_t = qt_pool.tile([128, LB * B], bf16, name=f"qt{ci}", tag=f"qt{ci}")
                # cast + relayout (b, l) -> (l, b) so each location pair is a
                # contiguous 128-column slice for matmul operands; spread
                # across gpsimd/DVE/ACT to hedge unmodeled strided-AP costs
                qdst = qt_t.rearrange("c (l b) -> c l b", b=B)
                qsrc = xt_t[:, ci].rearrange("c b l -> c l b")
                if ci < 2:
                    nc.gpsimd.tensor_copy(qdst, qsrc)
                elif ci == 2:
                    nc.vector.tensor_copy(qdst, qsrc)
                else:
                    nc.scalar.activation(qdst, qsrc, AF.Copy)
                qt.append(qt_t)

            for p in range(N_PAIR):
                # contiguous columns [p*128, (p+1)*128) = (j outer, b inner):
                # matmul M/N index = j*64+b  (pair-stacked)
                def pairT(tiles, ci, p=p):
                    return tiles[ci][:, p * 128 : (p + 1) * 128]

                # mm1: scores for the pair (plus ignored cross blocks)
                ps_s = ps_s_pool.tile([128, 128], f32)
                for ci in range(CCH):
                    nc.tensor.matmul(
                        ps_s,
                        pairT(qt, ci),
                        pairT(qt, ci),
                        start=(ci == 0),
                        stop=(ci == CCH - 1),
                    )

                # one full-tile exp; cross blocks are garbage but never read
                e = sm_pool.tile([128, 128], bf16, tag="e")
                nc.scalar.activation(e, ps_s, AF.Exp, scale=SCALE)

                # rowsums over the diagonal blocks only (per partition half)
                rs = sm_pool.tile([128, 1], f32, tag="rs")
                nc.vector.reduce_sum(rs[0:64], e[0:64, 0:64], axis=mybir.AxisListType.X)
                nc.vector.reduce_sum(
                    rs[64:128], e[64:128, 64:128], axis=mybir.AxisListType.X
                )
                inv = sm_pool.tile([128, 1], f32, tag="inv")
                nc.vector.reciprocal(inv, rs)

                # q_BC: transpose each qT chunk via identity matmul
                ps_t = ps_t_pool.tile([128, 512], f32)
                for ci in range(CCH):
                    nc.tensor.matmul(
                        ps_t[:, ci * 128 : (ci + 1) * 128],
                        pairT(qt, ci),
                        ident,
                        start=True,
                        stop=True,
                    )
                qbc = sm_pool.tile([128, 512], bf16, tag="qbc")
                if p % 2 == 0:
                    nc.vector.tensor_copy(qbc, ps_t)
                else:
                    nc.scalar.activation(qbc, ps_t, AF.Copy)

                # mm2: two row/col-tiled K=64 matmuls, one per location
                ps_m = ps_m_pool.tile([128, 512], f32)
                nc.tensor.matmul(
                    ps_m[0:64, :], e[0:64, 0:64], qbc[0:64, :],
                    start=True, stop=True, tile_position=(0, 0),
                )
                nc.tensor.matmul(
                    ps_m[64:128, :], e[64:128, 64:128], qbc[64:128, :],
                    start=True, stop=True, tile_position=(64, 64),
                )

                # scale rows by 1/rowsum while copying out of PSUM
                msg = sm_pool.tile([128, 512], bf16, tag="msg")
                nc.scalar.activation(msg, ps_m, AF.Copy, scale=inv)

                # transpose back to [c, (j, b)] and add into x tile (f32)
                ps_o = ps_o_pool.tile([128, CCH * 128], f32)
                for ci in range(CCH):
                    nc.tensor.matmul(
                        ps_o[:, ci * 128 : (ci + 1) * 128],
                        msg[:, ci * 128 : (ci + 1) * 128],
                        ident,
                        start=True,
                        stop=True,
                    )
                dst = xt_t[:, :, :, 2 * p : 2 * p + 2]  # [128, ci, b, j]
                src = ps_o.rearrange("c (ci j b) -> c ci b j", ci=CCH, j=2)
                nc.vector.tensor_add(dst, src, dst)

            for ci in range(CCH):
                nc.sync.dma_start(
                    out=y[:, ci * 128 : (ci + 1) * 128, l0 : l0 + LB].rearrange(
                        "b c l -> c b l"
                    ),
                    in_=xt_t[:, ci],
                )
    _hoist_extra_waits(nc)
    return nc


def _hoist_extra_waits(nc):
    """The 64B instruction encodings have room for only one embedded
    sem-wait, but Tile sometimes emits 2+ (foreign engine + self).  Splice
    same-engine NoOps (one wait each) before such instructions; the
    instruction keeps its last wait plus its sem updates."""
    import concourse.mybir as mybir

    n_fixed = 0
    for f in nc.m.functions:
        for blk in f.blocks:
            new_insts = []
            for inst in blk.instructions:
                si = inst.sync_info
                if si is not None and len(si.on_wait) > 1:
                    waits = list(si.on_wait)
                    for wi, w in enumerate(waits[:-1]):
                        nop = mybir.InstNoOp(
                            name=f"{inst.name}-wsp{wi}", ins=[], outs=[]
                        )
                        nop.engine = inst.engine
                        nop.sync_info = mybir.SyncInfo(on_wait=[w], on_update=[])
                        new_insts.append(nop)
                    inst.sync_info = mybir.SyncInfo(
                        on_wait=[waits[-1]], on_update=list(si.on_update)
                    )
                    n_fixed += 1
                new_insts.append(inst)
            if n_fixed:
                try:
                    blk.instructions = new_insts
                except Exception:
                    blk.instructions.clear()
                    blk.instructions.extend(new_insts)
    return n_fixed


_NC_CACHE = {}


def kernel(x: np.ndarray) -> np.ndarray:
    from concourse.bass_utils import run_bass_kernel_spmd

    assert x.shape == (B, C, L_FULL) and x.dtype == np.float32
    if "nc" not in _NC_CACHE:
        _NC_CACHE["nc"] = build_nc()
    nc = _NC_CACHE["nc"]

    in_maps = [
        {"x": np.ascontiguousarray(x[:, :, i * LS : (i + 1) * LS])}
        for i in range(N_CORES)
    ]
    res = run_bass_kernel_spmd(nc, in_maps, core_ids=list(range(N_CORES)))
    out = np.concatenate([res.results[i]["y"] for i in range(N_CORES)], axis=2)
    return out



# revision 3
# speedup vs baseline: 1.0183x; 1.0183x over previous
"""AttentionalPropagation kernel for Trainium2 (Bass/Tile), 8-core SPMD.

x: [B=64, C=512, L=4096] f32.  Per location l: self-attention over the B axis
(q=k=v, head dim C), out = x + msg.  Sharded over L: each of 8 cores handles
LS = 512 locations.

Design notes (v3 — DMA-bound; cost-model floor ~373us/core):
  - All DMA uses >=512B contiguous runs (l-chunks of 128 f32) -> full 360GB/s
    in the cost model (runs <512B pay a 2x latency multiplier).
  - Blocks of LB=128 locations; pairs (p, p+64) pack two 64-wide attention
    problems into 128-wide PE ops.  4 blocks/core, 64 pairs/block, groups of
    4 pairs share PSUM banks and batched vector ops.
  - scores = qT^T qT via 4 accumulating matmuls per pair (qt: [c, lo, j, b]
    bf16, cast+permuted from the f32 DMA staging).
  - softmax without per-column broadcasts: segmented DVE reduce into [128, 4]
    (per-partition-correct column per pair), reciprocal, then ONE
    scalar_tensor_tensor per pair: P_I = e * inv + I.  The +I folds the
    residual (+x) into mm2 for free.
  - PT = PE transpose of P_I (bf16 PSUM out), DVE-copied to SBUF at 2x rate.
    mm2: lhsT = qbc (PE transpose of qt pair slice), rhs = PT diag blocks,
    K=64 quadrant matmuls (tile_position (0,0)/(64,0)), out = x + msg.
  - ci-major output phases keep SBUF under the ~208KB/partition budget while
    preserving 512B out runs.
  - Software-pipelined emission: the serialized DMA resource sees
    [in(k,ci) ; out(k-1,ci)] x4 per block with each out's compute already
    queued one slot earlier, and block k's ci=0 output phase is interleaved
    into stage2(k) at group granularity so the DVE softmax chain never
    starves the DMA queue.
"""

import numpy as np

B, C, L_FULL, N_CORES = 64, 512, 4096, 8
LS = L_FULL // N_CORES  # 512 locations per core
LB = 128                # locations per block
N_BLK = LS // LB        # 4
HB = LB // 2            # 64 pairs per block; pair p = locations (p, p+HB)
N_GRP = HB // 4         # 16 groups of 4 pairs
CCH = C // 128          # 4 c-chunks
SCALE = 1.0 / float(C) ** 0.5


DEFAULT_CFG = dict(
    fp8=False,          # qt/qbc in fp8e4m3 (fails accuracy; keep False)
    qt_bufs=5,
    xt_bufs=4,
    emission="v3",      # unused (kept for sweep compat)
    rs_bf16=True,       # rowsum dtype bf16 (DVE 2x reduce)
    ptcopy="alt",       # dve | alt  (NOTE: gpsimd cannot access PSUM)
    ptcopy_tail="alt",  # ptcopy engine in spread (tail) blocks
    stt_pool=3,         # how many of the 4 stt per group go to gpsimd
    stt_pool_tail=3,    # same, for the spread (tail) block
    cast_eng="ADAA",    # per-bq engine: P=pool D=dve A=act
    qbc_mode="alt",     # alt: (g+ci)%2 ACT/DVE | ci0act
    y_mode="AD",        # y-copy engine by (g+ci) mod len (no P: PSUM)
    spread_first=False,  # apply tail spread treatment to block 0 too
    spread_last=True,   # spread last block's softmax chain
)


def build_nc(ls=LS, cfg=None):
    from contextlib import ExitStack

    import concourse.bass as bass
    import concourse.mybir as mybir
    from concourse.masks import make_identity
    from concourse.tile import TileContext

    f32 = mybir.dt.float32
    bf16 = mybir.dt.bfloat16
    fp8 = mybir.dt.float8e4
    AF = mybir.ActivationFunctionType
    AX = mybir.AxisListType
    ALU = mybir.AluOpType

    cfg = {**DEFAULT_CFG, **(cfg or {})}
    n_blk = ls // LB

    nc = bass.Bass()
    x = nc.dram_tensor("x", [B, C, ls], f32, kind="ExternalInput")
    y = nc.dram_tensor("y", [B, C, ls], f32, kind="ExternalOutput")

    with ExitStack() as ctx:
        tc = ctx.enter_context(TileContext(nc))
        const = ctx.enter_context(tc.tile_pool(name="const", bufs=1))
        xt_pool = ctx.enter_context(tc.tile_pool(name="xt", bufs=cfg["xt_bufs"]))
        qt_pool = ctx.enter_context(tc.tile_pool(name="qt", bufs=cfg["qt_bufs"]))
        e_pool = ctx.enter_context(tc.tile_pool(name="e", bufs=3))
        rs_pool = ctx.enter_context(tc.tile_pool(name="rs", bufs=3))
        inv_pool = ctx.enter_context(tc.tile_pool(name="inv", bufs=3))
        pi_pool = ctx.enter_context(tc.tile_pool(name="pi", bufs=4))
        pt_pool = ctx.enter_context(tc.tile_pool(name="pt", bufs=N_GRP + 1))
        qbc_pool = ctx.enter_context(tc.tile_pool(name="qbc", bufs=3))
        y_pool = ctx.enter_context(tc.tile_pool(name="y", bufs=2))
        ps_s_pool = ctx.enter_context(tc.tile_pool(name="ps_s", bufs=2, space="PSUM"))
        ps_t_pool = ctx.enter_context(tc.tile_pool(name="ps_t", bufs=2, space="PSUM"))
        ps_q_pool = ctx.enter_context(tc.tile_pool(name="ps_q", bufs=2, space="PSUM"))
        ps_y_pool = ctx.enter_context(tc.tile_pool(name="ps_y", bufs=2, space="PSUM"))

        qdt = fp8 if cfg["fp8"] else bf16
        ident = const.tile([128, 128], bf16)
        make_identity(nc, ident)
        if cfg["fp8"]:
            ident8 = const.tile([128, 128], fp8)
            make_identity(nc, ident8)
        else:
            ident8 = ident

        qt = {}       # (blk, ci) -> tile [128, HB(lo), 2(j), B] bf16
        pt_tiles = {}  # (blk, g) -> tile [128, 512] bf16
        y_tiles = {}   # (blk, ci) -> tile [128, B, LB] f32

        def emit_in(k, ci):
            l0 = k * LB
            qt[(k, ci)] = qt_pool.tile([128, HB, 2, B], qdt, name="qt")
            for bq in range(4):
                xt_ch = xt_pool.tile([128, 16, LB], f32, name="xtch")
                nc.sync.dma_start(
                    out=xt_ch,
                    in_=x[
                        bq * 16 : (bq + 1) * 16,
                        ci * 128 : (ci + 1) * 128,
                        l0 : l0 + LB,
                    ].rearrange("b c l -> c b l"),
                )
                # cast f32->bf16 with (b, l) -> (lo, j, b) permute (l = j*HB+lo)
                dst = qt[(k, ci)][:, :, :, bq * 16 : (bq + 1) * 16]
                src = xt_ch.rearrange("c b (j lo) -> c lo j b", j=2)
                ce = cfg["cast_eng"][bq]
                if ce == "A":
                    nc.scalar.activation(dst, src, AF.Copy)
                elif ce == "D":
                    nc.vector.tensor_copy(dst, src)
                else:
                    nc.gpsimd.tensor_copy(dst, src)

        def emit_out(k, ci):
            l0 = k * LB
            for bh in range(2):
                nc.sync.dma_start(
                    out=y[
                        bh * 32 : (bh + 1) * 32,
                        ci * 128 : (ci + 1) * 128,
                        l0 : l0 + LB,
                    ].rearrange("b c l -> c b l"),
                    in_=y_tiles[(k, ci)][:, bh * 32 : (bh + 1) * 32],
                )

        def emit_mm1(k, g):
            ps_s = ps_s_pool.tile([128, 512], f32)
            for pp in range(4):
                p = 4 * g + pp
                for ci in range(CCH):
                    nc.tensor.matmul(
                        ps_s[:, pp * 128 : (pp + 1) * 128],
                        qt[(k, ci)][:, p],
                        qt[(k, ci)][:, p],
                        start=(ci == 0),
                        stop=(ci == CCH - 1),
                    )
            return ps_s

        def emit_softmax(k, g, ps_s, spread=False):
            e = e_pool.tile([128, 512], bf16, name="e")
            nc.scalar.activation(e, ps_s, AF.Exp, scale=SCALE)
            rs = rs_pool.tile([128, 4], bf16 if cfg["rs_bf16"] else f32, name="rs")
            ev = e.rearrange("n (p q) -> n p q", q=128)
            with nc.allow_low_precision(reason="rowsum 2x; 64-term sums"):
                nc.vector.reduce_sum(rs[0:64], ev[0:64, :, 0:64], axis=AX.X)
                nc.vector.reduce_sum(
                    rs[64:128], ev[64:128, :, 64:128], axis=AX.X
                )
            inv = inv_pool.tile([128, 4], f32, name="inv")
            nc.vector.reciprocal(inv, rs)
            pi = pi_pool.tile([128, 512], bf16, name="pi")
            n_pool = cfg["stt_pool_tail"] if spread else cfg["stt_pool"]
            for pp in range(4):
                eng = nc.gpsimd if pp >= 4 - n_pool else nc.vector
                eng.scalar_tensor_tensor(
                    pi[:, pp * 128 : (pp + 1) * 128],
                    e[:, pp * 128 : (pp + 1) * 128],
                    inv[:, pp : pp + 1],
                    ident,
                    ALU.mult,
                    ALU.add,
                )
            return pi

        def emit_pt(k, g, pi, spread=False):
            ps_t = ps_t_pool.tile([128, 512], bf16)
            for pp in range(4):
                nc.tensor.transpose(
                    ps_t[:, pp * 128 : (pp + 1) * 128],
                    pi[:, pp * 128 : (pp + 1) * 128],
                    ident,
                )
            pt = pt_pool.tile([128, 512], bf16, name="pt")
            mode = cfg["ptcopy_tail"] if spread else cfg["ptcopy"]
            if mode == "pool":
                nc.gpsimd.tensor_copy(pt, ps_t)
            elif mode == "act" or (mode == "alt" and g % 2 == 1):
                nc.scalar.activation(pt, ps_t, AF.Copy)
            else:
                nc.vector.tensor_copy(pt, ps_t)
            pt_tiles[(k, g)] = pt

        def emit_qbc(k, ci, g):
            ps_q = ps_q_pool.tile([128, 512], qdt)
            for pp in range(4):
                nc.tensor.transpose(
                    ps_q[:, pp * 128 : (pp + 1) * 128],
                    qt[(k, ci)][:, 4 * g + pp],
                    ident8,
                )
            qbc = qbc_pool.tile([128, 512], qdt, name="qbc")
            act_it = (
                ci == 0 or (g + ci) % 2 == 0
            ) if cfg["qbc_mode"] == "ci0act" else ((g + ci) % 2 == 0)
            if act_it:
                nc.scalar.activation(qbc, ps_q, AF.Copy)
            else:
                nc.vector.tensor_copy(qbc, ps_q)
            return qbc

        def emit_mm2(k, ci, yv, g, qbc):
            pt = pt_tiles[(k, g)]
            ps_y = ps_y_pool.tile([128, 512], f32)
            for pp in range(4):
                for j in range(2):
                    c0 = pp * 128 + j * 64
                    nc.tensor.matmul(
                        ps_y[:, c0 : c0 + 64],
                        qbc[j * 64 : (j + 1) * 64, pp * 128 : (pp + 1) * 128],
                        pt[j * 64 : (j + 1) * 64, c0 : c0 + 64],
                        start=True,
                        stop=True,
                    )
            dst = yv[:, 4 * g : 4 * g + 4]
            src = ps_y.rearrange("c (pp j n) -> c pp j n", pp=4, j=2)
            ym = cfg["y_mode"]
            e_ch = ym[(g + ci) % len(ym)]
            if e_ch == "A":
                nc.scalar.activation(dst, src, AF.Copy)
            elif e_ch == "D":
                nc.vector.tensor_copy(dst, src)
            else:
                nc.gpsimd.tensor_copy(dst, src)

        def new_y(k, ci):
            y_ci = y_pool.tile([128, B, LB], f32, name="yci")
            y_tiles[(k, ci)] = y_ci
            return y_ci.rearrange("c b (jj ll) -> c ll jj b", jj=2)

        def emit_cip(k, ci):
            """Full output phase for (k, ci) minus its out-DMA."""
            yv = new_y(k, ci)
            pend = []
            for g in range(N_GRP):
                qbc = emit_qbc(k, ci, g)
                pend.append((g, qbc))
                if len(pend) > 1:
                    emit_mm2(k, ci, yv, *pend.pop(0))
            for gg, qbc in pend:
                emit_mm2(k, ci, yv, gg, qbc)

        def emit_phase_b(k, spread=False, extra_ci1=False):
            """stage2(k) interleaved with cip(k, 3) at group granularity
            (ci=3 so the qt pool's FIFO release order matches allocation
            order).  extra_ci1: also interleave cip(k, 0) one lag later."""
            yv = new_y(k, 3)
            yv1 = new_y(k, 0) if extra_ci1 else None
            sm_pend = []   # (g, pi) awaiting PT
            c0_pend = []   # (g, qbc) awaiting mm2 ci0
            c1_pend = []   # (g, qbc) awaiting mm2 ci1
            done0 = []     # groups whose ci0 mm2 is emitted

            def step_tail():
                if len(c0_pend) > 1:
                    gg, qbc = c0_pend.pop(0)
                    emit_mm2(k, 3, yv, gg, qbc)
                    done0.append(gg)
                if extra_ci1 and done0:
                    gg = done0.pop(0)
                    c1_pend.append((gg, emit_qbc(k, 0, gg)))
                if extra_ci1 and len(c1_pend) > 1:
                    emit_mm2(k, 0, yv1, *c1_pend.pop(0))

            for g in range(N_GRP):
                ps_s = emit_mm1(k, g)
                sm_pend.append((g, emit_softmax(k, g, ps_s, spread)))
                if len(sm_pend) > 2:
                    gg, pi = sm_pend.pop(0)
                    emit_pt(k, gg, pi, spread)
                    c0_pend.append((gg, emit_qbc(k, 3, gg)))
                step_tail()
            for gg, pi in sm_pend:
                emit_pt(k, gg, pi, spread)
                c0_pend.append((gg, emit_qbc(k, 3, gg)))
                step_tail()
            while c0_pend:
                gg, qbc = c0_pend.pop(0)
                emit_mm2(k, 3, yv, gg, qbc)
                done0.append(gg)
            if extra_ci1:
                for gg in done0:
                    c1_pend.append((gg, emit_qbc(k, 0, gg)))
                    if len(c1_pend) > 1:
                        emit_mm2(k, 0, yv1, *c1_pend.pop(0))
                for gg, qbc in c1_pend:
                    emit_mm2(k, 0, yv1, gg, qbc)

        for k in range(n_blk):
            emit_in(k, 0)
            if k > 0:
                emit_out(k - 1, 3)
                emit_cip(k - 1, 0)
            emit_in(k, 1)
            if k > 0:
                emit_out(k - 1, 0)
                emit_cip(k - 1, 1)
            emit_in(k, 2)
            if k > 0:
                emit_out(k - 1, 1)
                emit_cip(k - 1, 2)
            emit_in(k, 3)
            if k > 0:
                emit_out(k - 1, 2)
            emit_phase_b(
                k,
                spread=(cfg["spread_last"] and k == n_blk - 1)
                or (cfg["spread_first"] and k == 0),
                extra_ci1=(cfg.get("tail_extra") and k == n_blk - 1),
            )
        kl = n_blk - 1
        emit_out(kl, 3)
        if cfg.get("tail_extra"):
            emit_out(kl, 0)
            for ci in (1, 2):
                emit_cip(kl, ci)
                emit_out(kl, ci)
        else:
            for ci in range(CCH - 1):
                emit_cip(kl, ci)
                emit_out(kl, ci)

    _hoist_extra_waits(nc)
    return nc


def _hoist_extra_waits(nc):
    """The 64B instruction encodings have room for only one embedded
    sem-wait, but Tile sometimes emits 2+ (foreign engine + self).  Splice
    same-engine NoOps (one wait each) before such instructions; the
    instruction keeps its last wait plus its sem updates."""
    import concourse.mybir as mybir

    n_fixed = 0
    for f in nc.m.functions:
        for blk in f.blocks:
            new_insts = []
            for inst in blk.instructions:
                si = inst.sync_info
                if si is not None and len(si.on_wait) > 1:
                    waits = list(si.on_wait)
                    for wi, w in enumerate(waits[:-1]):
                        nop = mybir.InstNoOp(
                            name=f"{inst.name}-wsp{wi}", ins=[], outs=[]
                        )
                        nop.engine = inst.engine
                        nop.sync_info = mybir.SyncInfo(on_wait=[w], on_update=[])
                        new_insts.append(nop)
                    inst.sync_info = mybir.SyncInfo(
                        on_wait=[waits[-1]], on_update=list(si.on_update)
                    )
                    n_fixed += 1
                new_insts.append(inst)
            if n_fixed:
                try:
                    blk.instructions = new_insts
                except Exception:
                    blk.instructions.clear()
                    blk.instructions.extend(new_insts)
    return n_fixed


_NC_CACHE = {}


def kernel(x: np.ndarray) -> np.ndarray:
    from concourse.bass_utils import run_bass_kernel_spmd

    assert x.shape == (B, C, L_FULL) and x.dtype == np.float32
    if "nc" not in _NC_CACHE:
        _NC_CACHE["nc"] = build_nc()
    nc = _NC_CACHE["nc"]

    in_maps = [
        {"x": np.ascontiguousarray(x[:, :, i * LS : (i + 1) * LS])}
        for i in range(N_CORES)
    ]
    res = run_bass_kernel_spmd(nc, in_maps, core_ids=list(range(N_CORES)))
    out = np.concatenate([res.results[i]["y"] for i in range(N_CORES)], axis=2)
    return out
